# revision 1
# baseline (speedup 1.0000x reference)
"""Trainium2 Bass kernel: Performer (linear) attention + in/out projections.

Problem nn_LinearPerformerAttention_6717328851263:
  x:(4,4096,1024) f32, w_qkv:(1024,3072), proj_matrix:(16,64,256),
  w_out:(1024,1024), b_out:(1024,)

  qkv = x @ w_qkv ; split q,k,v ; per (b,h): q_proj=elu1(q@P_h), k_proj=elu1(k@P_h)
  kv = k_proj^T v ; k_sum = sum_n k_proj ; attn = (q_proj @ kv) / (q_proj@k_sum)
  out = attn @ w_out + b_out

Sharding over 8 cores: core c -> (batch b=c//2, head-group g=c%2: 8 of 16 heads).
Each core computes partial y_c = attn(b, heads_g) @ w_out[512g:512g+512, :].
Host gather: out[b] = y_(b,0) + y_(b,1) + b_out.

v3: all matmul streams bf16 (PSUM stays fp32; host pre-casts inputs), qT kept
resident in SBUF (no DRAM spill), and both passes software-pipelined so the
elu1 elementwise latency hides under the big PE streams:
  pass A, iteration g: [qT(g) kT(g)] then v(g)/kproj(g) quads interleaved with
    kv(g-1) quads - kproj(g)'s elu flows on scalar/vector while the PE streams
    qkv(g+1).  kv state accumulates per (head-pair, group) in PSUM, folded to
    fp32 SBUF.
  fixup: kvT -> transpose -> pair-packed kvS[h][s] [128,128] bf16 and ksr
    (k_sum replicated over the head's 64 cols) for the denominator.
  pass B, per group: qproj+elu for pairs 0,1 up front; per pair hp: attn+denom
    matmuls, z = recip(denom), att = attn*z, then qproj+elu(pair hp+2); y
    matmuls last (their stream covers the next group's elu latency).
elu1(x) = elu(x)+1 = min(exp(x),1) + relu(x), emitted in two variants to
balance engines: S-heavy (Exp+Relu on scalar, one stt on vector) and V-heavy
(Exp on scalar, relu+1 and min on vector), ratio tunable per pass.
"""

import numpy as np
from contextlib import ExitStack

import ml_dtypes
import concourse.bass as bass
import concourse.bacc as bacc
import concourse.tile as tile
from concourse import mybir
from concourse.bass_utils import run_bass_kernel_spmd
from concourse.masks import make_identity

FP32 = mybir.dt.float32
BF16 = mybir.dt.bfloat16
AL = mybir.AluOpType
AF = mybir.ActivationFunctionType

B, SEQ, D = 4, 4096, 1024
H, HD, F = 16, 64, 256
HPC = 8            # heads per core
DH = HPC * HD      # 512 head-space dims per core
P = 128
NCORES = 8

# elu1 variant pattern (scalar-heavy vs vector-heavy), per pass
SHEAVY_A = (True, False)                                  # 8/16 S-heavy
SHEAVY_B = (True, True, False, True)     # 6/8 of the 1024-wide B tiles


def _emit(tc, n, xT, wq, wk, wv, proj, wout, y):
    nc = tc.nc
    NG = n // 512       # token groups
    TPG = 4             # 128-token tiles per group

    def emit_elu(c_ps, out, pool, s_heavy, tag, w=512):
        """elu1 = min(exp(c),1) + relu(c), c in PSUM fp32 -> bf16 `out`."""
        e = pool.tile([P, w], BF16, tag=f"{tag}E", name=f"{tag}E")
        r = pool.tile([P, w], BF16, tag=f"{tag}R", name=f"{tag}R")
        nc.scalar.activation(e, c_ps, AF.Exp)
        if s_heavy:
            nc.scalar.activation(r, c_ps, AF.Relu)
            nc.vector.scalar_tensor_tensor(
                out, in0=e, scalar=1.0, in1=r, op0=AL.min, op1=AL.add)
        else:
            nc.vector.tensor_scalar(r, c_ps, 0.0, 1.0, op0=AL.max, op1=AL.add)
            nc.vector.tensor_tensor(out=out, in0=e, in1=r, op=AL.min)

    ctx = ExitStack()
    with ctx:
        const = ctx.enter_context(tc.tile_pool(name="const", bufs=1))

        ident = const.tile([P, P], FP32, tag="ident", name="ident")
        make_identity(nc, ident)
        ones16 = const.tile([P, P], BF16, tag="ones16", name="ones16")
        nc.vector.memset(ones16, 1.0)
        ones32 = const.tile([P, P], FP32, tag="ones32", name="ones32")
        nc.vector.memset(ones32, 1.0)

        # proj, pair-packed [128, 256]: head 2i at partitions 0:64, head
        # 2i+1 at 64:128 (so lhsT/rhs partition bases always match).
        proj_all = const.tile([P, 4, F], BF16, tag="projp", name="projp")
        proj_pair = [proj_all[:, i, :] for i in range(4)]
        # (proj DMA is issued in pass A, after wk/wo - it is not needed
        # until the first kproj, and issuing it first would delay wq)
        # w_out, needed only in pass B but loaded early while DMA is free
        wo_all = const.tile([P, 4, D], BF16, tag="wo", name="wo")
        wo_sb = [wo_all[:, s, :] for s in range(4)]

        # attn lhsT, zero-padded to M=128 so a head pair accumulates into one
        # [128,512] PSUM tile (matmul outputs must start at partition 0).
        kvS = [[const.tile([P, P], BF16, tag=f"kvS{h}_{s}", name=f"kvS{h}_{s}")
                for s in range(2)] for h in range(HPC)]
        # denominator lhsT: ksr[h][s] cols (h%2)*64.. replicate k_sum_h,
        # rest zero -> pair denominators land on the matching partitions of
        # one PSUM tile (z broadcast for free)
        ksr = [[const.tile([P, P], BF16, tag=f"ksr{h}_{s}", name=f"ksr{h}_{s}")
                for s in range(2)] for h in range(HPC)]
        for h in range(HPC):
            ho = HD - (h % 2) * HD
            for s in range(2):
                nc.vector.memset(kvS[h][s][:, ho:ho + HD], 0.0)
                nc.vector.memset(ksr[h][s][:, ho:ho + HD], 0.0)

        # qT resident in SBUF across both passes: 4 dh-slabs x [128, n] bf16
        qt_sb = [const.tile([P, n], BF16, tag=f"qt{s}", name=f"qt{s}")
                 for s in range(4)]
        # warm-start tiles: group 0's qP for pairs 0/1, computed during the
        # pass-A epilogue so pass B's first attn doesn't wait on its prologue
        qP_warm = [const.tile([P, 1024], BF16, tag=f"qPw{i}", name=f"qPw{i}")
                   for i in range(4)]

        # ---------------- pass A (group-pipelined) ----------------
        with ExitStack() as actx:
            kvaccp = actx.enter_context(tc.tile_pool(name="kvaccp", bufs=1))
            kv_acc = [kvaccp.tile([HD + 1, 512], FP32, tag=f"kva{i}", name=f"kva{i}")
                      for i in range(4)]
            # Weights in single [128, 8, 512] tiles - one DMA issue each
            # (DMA issue costs ~630ns of queue time; 8 small ones serialize).
            # wq first (sync queue) so the first qT matmuls wait only on
            # wq + xt(0) (prefetched below, before wk); wv goes on the
            # scalar DMA queue to run in parallel.
            wpool = actx.enter_context(tc.tile_pool(name="wpool", bufs=1))
            wq_all = wpool.tile([P, 8, DH], BF16, tag="wq", name="wq")
            wk_all = wpool.tile([P, 8, DH], BF16, tag="wk", name="wk")
            wv_all = wpool.tile([P, 8, DH], BF16, tag="wv", name="wv")
            wq_sb = [wq_all[:, s, :] for s in range(8)]
            wk_sb = [wk_all[:, s, :] for s in range(8)]
            wv_sb = [wv_all[:, s, :] for s in range(8)]
            nc.sync.dma_start(out=wq_all, in_=wq.rearrange("(s p) m -> p s m", p=P))

            xtpool = actx.enter_context(tc.tile_pool(name="xtpool", bufs=2))
            ktpool = actx.enter_context(tc.tile_pool(name="ktpool", bufs=2))
            vpool = actx.enter_context(tc.tile_pool(name="vpool", bufs=2))
            elupool = actx.enter_context(tc.tile_pool(name="elupool", bufs=4))
            kppool = actx.enter_context(tc.tile_pool(name="kppool", bufs=20))
            mmps = actx.enter_context(tc.tile_pool(name="mmps", bufs=2, space="PSUM"))
            kpps = actx.enter_context(tc.tile_pool(name="kpps", bufs=2, space="PSUM"))
            kvps = actx.enter_context(tc.tile_pool(name="kvps", bufs=2, space="PSUM"))

            xT_v = xT.rearrange("(s p) m -> p s m", p=P)

            # prefetch xt(0) FIRST on the scalar DMA queue (needed before
            # wv), in parallel with the wq load on the sync queue
            xt0 = xtpool.tile([P, 8, 512], BF16, tag="xt", name="xt")
            nc.scalar.dma_start(out=xt0, in_=xT_v[:, :, 0:512])
            nc.scalar.dma_start(out=wv_all, in_=wv.rearrange("(s p) m -> p s m", p=P))
            nc.sync.dma_start(out=wk_all, in_=wk.rearrange("(s p) m -> p s m", p=P))
            nc.sync.dma_start(out=wo_all, in_=wout.rearrange("(s p) m -> p s m", p=P))
            nc.sync.dma_start(out=proj_all, in_=proj.rearrange("(i p) f -> p i f", p=P))

            # per-group state carried one iteration (group g processed for
            # kv in iteration g+1)
            kt_all = [None] * NG
            vone_all = [None] * NG
            kP_all = [[None] * 16 for _ in range(NG)]   # (hp, tp, h%2) -> idx

            def kv_quad(g, hp):
                """kv accumulation for (group g, head pair hp): 8 matmuls
                into one PSUM tile + fold into kv_acc."""
                vone = vone_all[g]
                kv_ps = kvps.tile([HD + 1, 512], FP32, tag="kvg", name="kvg")
                nmm = 0
                for tp in range(2):
                    for hh in range(2):
                        kP = kP_all[g][hp * 4 + tp * 2 + hh]
                        for ti in range(2):
                            t = tp * 2 + ti
                            nc.tensor.matmul(
                                kv_ps[:, hh * F:(hh + 1) * F],
                                lhsT=(vone[:, t, 2 * hp + hh, :]),
                                rhs=(kP[:, ti * F:(ti + 1) * F]),
                                start=(nmm == 0), stop=(nmm == 7),
                                skip_group_check=True)
                            nmm += 1
                if g == 0:
                    nc.vector.tensor_copy(kv_acc[hp], kv_ps)
                else:
                    nc.vector.tensor_tensor(
                        out=kv_acc[hp], in0=kv_ps, in1=kv_acc[hp], op=AL.add)

            for g in range(NG + 1):
                if g < NG:
                    g0 = g * 512
                    if g == 0:
                        xt = xt0
                    else:
                        xt = xtpool.tile([P, 8, 512], BF16, tag="xt", name="xt")
                        nc.sync.dma_start(out=xt, in_=xT_v[:, :, g0:g0 + 512])

                    # qT -> resident SBUF (evict: scalar)
                    for fs in range(4):
                        ps = mmps.tile([P, 512], FP32, tag="mm", name="mm")
                        for s in range(8):
                            nc.tensor.matmul(
                                ps, lhsT=(wq_sb[s][:, fs * P:(fs + 1) * P]),
                                rhs=(xt[:, s, :]), start=(s == 0), stop=(s == 7))
                        nc.scalar.copy(qt_sb[fs][:, g0:g0 + 512], ps)

                    # kT (evict: scalar)
                    kt_sb = [ktpool.tile([P, 512], BF16, tag=f"kt{fs}", name=f"kt{fs}")
                             for fs in range(4)]
                    kt_all[g] = kt_sb
                    for fs in range(4):
                        ps = mmps.tile([P, 512], FP32, tag="mm", name="mm")
                        for s in range(8):
                            nc.tensor.matmul(
                                ps, lhsT=(wk_sb[s][:, fs * P:(fs + 1) * P]),
                                rhs=(xt[:, s, :]), start=(s == 0), stop=(s == 7))
                        nc.scalar.copy(kt_sb[fs], ps)

                    # v with ones column (evict: vector), interleaved with
                    # kproj quads (this group) and kv quads (previous group)
                    vone = vpool.tile([P, TPG, HPC, HD + 1], BF16, tag="vone", name="vone")
                    vone_all[g] = vone
                    nc.vector.tensor_copy(
                        vone[:, :, :, HD],
                        ones16[:, 0:TPG * HPC].rearrange("p (t h) -> p t h", t=TPG))

                for j in range(TPG):
                    if g < NG:
                        t = j
                        ps = mmps.tile([P, 512], FP32, tag="mm", name="mm")
                        for s in range(8):
                            nc.tensor.matmul(
                                ps, lhsT=(xt[:, s, t * P:(t + 1) * P]),
                                rhs=(wv_sb[s]), start=(s == 0), stop=(s == 7))
                        nc.vector.tensor_copy(
                            vone_all[g][:, t, :, 0:HD],
                            ps.rearrange("p (h e) -> p h e", h=HPC))

                        # kproj quads for head pair hp=j: even/odd heads at
                        # partition bases 0/64 interleaved (disjoint PE row
                        # groups overlap); elu output lands directly in the
                        # long-lived kP tile used by next iteration's kv quad
                        hp = j
                        for tp in range(2):
                            cps = {hh: kpps.tile([P, 512], FP32, tag=f"kp{hh}",
                                                 name=f"kp{hh}")
                                   for hh in range(2)}
                            for ti in range(2):
                                t2 = tp * 2 + ti
                                for hh in range(2):
                                    hb = hh * HD
                                    nc.tensor.matmul(
                                        cps[hh][:, ti * F:(ti + 1) * F],
                                        lhsT=(kt_all[g][hp][hb:hb + HD,
                                                            t2 * P:(t2 + 1) * P]),
                                        rhs=(proj_pair[hp][hb:hb + HD, :]),
                                        start=True, stop=True)
                            for hh in range(2):
                                s_heavy = SHEAVY_A[(tp * 2 + hh) % len(SHEAVY_A)]
                                kP = kppool.tile([P, 512], BF16, tag="kP", name="kP")
                                emit_elu(cps[hh], kP, elupool, s_heavy, "k")
                                kP_all[g][hp * 4 + tp * 2 + hh] = kP

                    if g >= 1:
                        kv_quad(g - 1, j)
                        if g == NG:
                            # kv fixup for head pair j: kvT -> kvS/ksr,
                            # interleaved with the epilogue kv quads so the
                            # scalar/vector copies overlap remaining PE work
                            kvt_sb = kv_acc[j]
                            for jj in range(2):   # head h = 2j + jj
                                h = 2 * j + jj
                                hb = jj * HD
                                for s in range(2):   # F slab
                                    tp = mmps.tile([P, HD + 1], FP32,
                                                   tag="mm", name="tps")
                                    nc.tensor.transpose(
                                        tp,
                                        kvt_sb[:, jj * F + s * P:
                                               jj * F + (s + 1) * P],
                                        ident[0:HD + 1, 0:HD + 1])
                                    nc.vector.tensor_copy(
                                        kvS[h][s][:, hb:hb + HD], tp[:, 0:HD])
                                    # k_sum column replicated over the head's
                                    # 64 cols via a stride-0 (broadcast) read
                                    # (vector: the epilogue is scalar-bound)
                                    nc.vector.tensor_copy(
                                        ksr[h][s][:, hb:hb + HD],
                                        tp[:, HD:HD + 1].broadcast_to([P, HD]))
                            if j < 2:
                                # warm-start: group 0's qproj+elu for pair j
                                # (uses pass-A pools; consumed by pass B)
                                for hh in range(2):
                                    hb2 = hh * HD
                                    for s in range(2):
                                        cw = kpps.tile([P, 512], FP32,
                                                       tag=f"kp{hh}",
                                                       name=f"kp{hh}")
                                        nc.tensor.matmul(
                                            cw,
                                            lhsT=(proj_pair[j][hb2:hb2 + HD,
                                                               s * P:(s + 1) * P]),
                                            rhs=(qt_sb[j][hb2:hb2 + HD, 0:512]),
                                            start=True, stop=True)
                                        emit_elu(
                                            cw,
                                            qP_warm[2 * j + hh][:, s * 512:
                                                                (s + 1) * 512],
                                            elupool,
                                            SHEAVY_A[(hh * 2 + s) % len(SHEAVY_A)],
                                            "k")

        # ---------------- pass B (pair-pipelined) ----------------
        with ExitStack() as bctx:
            qppool = bctx.enter_context(tc.tile_pool(name="qppool", bufs=4))
            qPpool = bctx.enter_context(tc.tile_pool(name="qPpool", bufs=3))
            attpool = bctx.enter_context(tc.tile_pool(name="attpool", bufs=2))
            zpool = bctx.enter_context(tc.tile_pool(name="zpool", bufs=2))
            ypool = bctx.enter_context(tc.tile_pool(name="ypool", bufs=3))
            qpps = bctx.enter_context(tc.tile_pool(name="qpps", bufs=2, space="PSUM"))
            atps = bctx.enter_context(tc.tile_pool(name="atps", bufs=1, space="PSUM"))
            yps = bctx.enter_context(tc.tile_pool(name="yps", bufs=1, space="PSUM"))

            qP_next = [None, None]   # pairs 0/1 of group g, made in g-1
            for g in range(NG):
                g0 = g * 512

                qP_pairs = [None] * 4

                def qproj_pair(hp, goff, dest, di):
                    """qproj matmuls + elu for head pair hp of the group at
                    token offset goff.  Both F-slabs of one head share a
                    [128,1024] 2-bank PSUM tile so the elu runs as
                    1024-wide ops (halved instruction count)."""
                    pss = {hh: qpps.tile([P, 1024], FP32, tag="qp", name="qp")
                           for hh in range(2)}
                    for s in range(2):
                        for hh in range(2):   # even/odd interleave (PE rows)
                            hb = hh * HD
                            nc.tensor.matmul(
                                pss[hh][:, s * 512:(s + 1) * 512],
                                lhsT=(proj_pair[hp][hb:hb + HD,
                                                    s * P:(s + 1) * P]),
                                rhs=(qt_sb[hp][hb:hb + HD, goff:goff + 512]),
                                start=True, stop=True,
                                skip_group_check=True)
                    qPs = []
                    for hh in range(2):
                        s_heavy = SHEAVY_B[(hp * 2 + hh) % len(SHEAVY_B)]
                        qP = qPpool.tile([P, 1024], BF16, tag=f"qP{hh}",
                                         name=f"qP{hh}")
                        emit_elu(pss[hh], qP, qppool, s_heavy, "q", w=1024)
                        qPs.append((hh, qP))
                    dest[di] = qPs

                att_sb = [attpool.tile([P, 512], BF16, tag=f"att{i}", name=f"att{i}")
                          for i in range(4)]

                if g == 0:
                    qP_pairs[0] = [(0, qP_warm[0]), (1, qP_warm[1])]
                    qP_pairs[1] = [(0, qP_warm[2]), (1, qP_warm[3])]
                else:
                    qP_pairs[0] = qP_next[0]
                    qP_pairs[1] = qP_next[1]
                for hp in range(4):
                    aps = atps.tile([P, 512], FP32, tag="at", name="aps")
                    dps = atps.tile([P, 512], FP32, tag="dn", name="dn")
                    nmm = 0
                    for s in range(2):
                        for (hh, qP) in qP_pairs[hp]:
                            h = 2 * hp + hh
                            nc.tensor.matmul(
                                aps, lhsT=(kvS[h][s]),
                                rhs=(qP[:, s * 512:(s + 1) * 512]),
                                start=(nmm == 0), stop=(nmm == 3),
                                skip_group_check=True)
                            nc.tensor.matmul(
                                dps, lhsT=(ksr[h][s]),
                                rhs=(qP[:, s * 512:(s + 1) * 512]),
                                start=(nmm == 0), stop=(nmm == 3),
                                skip_group_check=True)
                            nmm += 1
                    zb = zpool.tile([P, 512], FP32, tag="zb", name="zb")
                    nc.vector.reciprocal_approx_fast(zb, dps)
                    nc.vector.tensor_tensor(
                        out=att_sb[hp], in0=aps, in1=zb, op=AL.mult)
                    if hp + 2 < 4:
                        qproj_pair(hp + 2, g0, qP_pairs, hp + 2)

                # next group's first two qproj pairs BEFORE the y stream -
                # their elu latency hides under the 32 y matmuls
                if g + 1 < NG:
                    qproj_pair(0, g0 + 512, qP_next, 0)
                    qproj_pair(1, g0 + 512, qP_next, 1)

                # y = attnT^^T @ w_out; both o-halves land in one 2-bank
                # PSUM tile -> a single evict + DMA per token tile
                for t in range(TPG):
                    pso = yps.tile([P, 1024], FP32, tag="yy", name="yy")
                    for s in range(4):
                        for o in range(2):
                            nc.tensor.matmul(
                                pso[:, o * 512:(o + 1) * 512],
                                lhsT=(att_sb[s][:, t * P:(t + 1) * P]),
                                rhs=(wo_sb[s][:, o * 512:(o + 1) * 512]),
                                start=(s == 0), stop=(s == 3),
                                skip_group_check=True)
                    y_sb = ypool.tile([P, 1024], BF16, tag="ysb", name="ysb")
                    if t % 2 == 0:
                        nc.scalar.copy(y_sb, pso)
                    else:
                        nc.vector.tensor_copy(y_sb, pso)
                    # store via the sync queue - idle in pass B, and a DMA
                    # issue on the scalar queue would delay the elu ACTs
                    nc.sync.dma_start(
                        out=y[g0 + t * P: g0 + (t + 1) * P, :], in_=y_sb)


def build(n=SEQ):
    nc = bacc.Bacc("TRN2", target_bir_lowering=False, debug=False,
                   enable_asserts=False)
    xT = nc.declare_dram_parameter("xT", [D, n], BF16, isOutput=False)
    wq = nc.declare_dram_parameter("wq", [D, DH], BF16, isOutput=False)
    wk = nc.declare_dram_parameter("wk", [D, DH], BF16, isOutput=False)
    wv = nc.declare_dram_parameter("wv", [D, DH], BF16, isOutput=False)
    proj = nc.declare_dram_parameter("proj", [DH, F], BF16, isOutput=False)
    wout = nc.declare_dram_parameter("wout", [DH, D], BF16, isOutput=False)
    y = nc.declare_dram_parameter("y", [n, D], BF16, isOutput=True)
    with tile.TileContext(nc) as tc:
        _emit(tc, n, xT, wq, wk, wv, proj, wout, y)
    nc.finalize()
    return nc


def make_in_maps(x, w_qkv, proj_matrix, w_out):
    bf = ml_dtypes.bfloat16
    x = np.asarray(x, np.float32)
    w_qkv = np.asarray(w_qkv, np.float32).astype(bf)
    proj_matrix = np.asarray(proj_matrix, np.float32).astype(bf)
    w_out = np.asarray(w_out, np.float32).astype(bf)
    in_maps = []
    for c in range(NCORES):
        b, g = c // 2, c % 2
        in_maps.append({
            "xT": np.ascontiguousarray(x[b].T.astype(bf)),
            "wq": np.ascontiguousarray(w_qkv[:, DH * g:DH * (g + 1)]),
            "wk": np.ascontiguousarray(w_qkv[:, D + DH * g:D + DH * (g + 1)]),
            "wv": np.ascontiguousarray(w_qkv[:, 2 * D + DH * g:2 * D + DH * (g + 1)]),
            "proj": np.ascontiguousarray(
                proj_matrix[HPC * g:HPC * (g + 1)].reshape(DH, F)),
            "wout": np.ascontiguousarray(w_out[DH * g:DH * (g + 1), :]),
        })
    return in_maps


_NC_CACHE = {}


def get_nc(n=SEQ):
    if n not in _NC_CACHE:
        _NC_CACHE[n] = build(n)
    return _NC_CACHE[n]


def _install_ntff_hook_shim():
    """The agent image's antenv lacks axon_hooks; recreate it so
    run_bass_kernel_spmd(trace=True) can capture NTFF profiles."""
    import sys
    import types
    try:
        from antenv.axon_hooks import get_axon_ntff_profile_hook  # noqa: F401
        return True
    except ImportError:
        pass
    try:
        from trn_agent_boot.trn_boot import _ntff_profile_via_ctypes
        import antenv
        mod = types.ModuleType("antenv.axon_hooks")
        mod._hook = _ntff_profile_via_ctypes("/opt/axon/libaxon_pjrt.so")
        mod.set_axon_ntff_profile_hook = lambda h: setattr(mod, "_hook", h)
        mod.get_axon_ntff_profile_hook = lambda: mod._hook
        sys.modules["antenv.axon_hooks"] = mod
        antenv.axon_hooks = mod
        return True
    except Exception as e:  # profiling is best-effort
        print(f"ntff hook shim failed: {e}")
        return False


def run(x, w_qkv, proj_matrix, w_out, b_out, trace=False, **kw):
    if trace:
        _install_ntff_hook_shim()
    nc = get_nc(SEQ)
    in_maps = make_in_maps(x, w_qkv, proj_matrix, w_out)
    res = run_bass_kernel_spmd(nc, in_maps, list(range(NCORES)),
                               trace=trace, **kw)
    b_out = np.asarray(b_out, np.float32)
    out = np.empty((B, SEQ, D), np.float32)
    for b in range(B):
        out[b] = res.results[2 * b]["y"].astype(np.float32) \
            + res.results[2 * b + 1]["y"].astype(np.float32) \
            + b_out[None, :]
    return out, res


def kernel(x, w_qkv, proj_matrix, w_out, b_out):
    out, _ = run(x, w_qkv, proj_matrix, w_out, b_out)
    return out

